# revision 26
# baseline (speedup 1.0000x reference)
"""Performer attention (causal, kernelized) — Trainium2 Bass kernel, v3.

Two launches on 8 cores:

  A) seq-sharded prep: core j owns 256 sequence positions and computes, for
     ALL 8 heads at once: kh (scaled k-projection), the LayerNorm-folded and
     scaled/biased q-projection qh, the v-projection in seq-major layout,
     and the local stabilizer max(h_k).  Each (position, head) projection is
     computed exactly once fleet-wide.

  B) head-sharded attention: core h owns head h end-to-end: Performer
     feature maps, the causal chunked prefix scan, output normalization and
     its row-block of the FC (W_fc row-sharded; host sums partials and adds
     bias + residual).  The exact global k_stab (host max over the 8 phase-A
     stabs) is folded into the k-feature exp bias — no approximation.

All big matmuls use float32r (4x PE throughput at free >= 256, ~2e-4
relative error; end-to-end max-rel stays ~1e-3).  Algebra notes (validated
against the reference):
  - q LayerNorm folded: Wq_eff = diag(gamma) Wq * scale, bias cq = beta@Wq*scale,
    applied to (q - mu) * rstd with rstd = exp(-0.5 ln(var + eps)).
  - exp(h_q + (proj_q - h_q)) == exp(proj_q): q-side stabilizer cancels.
  - k feature: exp(proj_k + h_k - k_stab) via the augmented contraction
    [kh; kh^2] . [rf^T; -0.5] plus a per-partition bias of -k_stab in the exp.
  - +KERNEL_EPS becomes extra features: q~ rows 266/267 = (sum_m exp_q + m*eps,
    eps); k~ cols 266/267 = (eps, sum_m exp_k); the global 1/sqrt(m) cancels
    except 1/c^2 folded into W_fc.
  - causal prefix scan chunked at C=128 with states per chunk-PAIR: the
    in-pair cross term (keys of even chunk x queries of odd chunk) rides in a
    [128 x 256] pair-attention block at full fp32r speed.
  - the non-causal normalizer d = q~ . z (z = column sums of k~) is computed
    from a separate z accumulation: o columns 64/66 carry D and d; no state
    column fixups needed.
  - the reference's |d|<=1e-6 guard is dead for any realistic data (d ~ 1e3+)
    and is omitted.
"""

import sys
for _p in ("/opt/trn_rl_repo", "/root/.axon_site/_ro/trn_rl_repo"):
    if _p not in sys.path:
        sys.path.append(_p)

import numpy as np

import concourse.bass as bass
from concourse import bacc
import concourse.mybir as mybir
import concourse.tile as tile
from concourse.bass import ts, ds
from concourse.bass_utils import run_bass_kernel_spmd

F32 = mybir.dt.float32
F32R = mybir.dt.float32r
NC = 8
N = 2048
D_MODEL = 512
D_K = 64
D_V = 64
M = 266
C = 128
NCH = N // C            # 16 chunks
NPAIR = NCH // 2        # 8 chunk pairs
SLA = N // NC           # 256 seq positions per phase-A core
NSL = 4                 # 512-wide slices of the full sequence
SL = 512
KERNEL_EPS = 1e-4
LN_EPS = 1e-6
SCALE = float(D_MODEL) ** (-0.25)
EXP = mybir.ActivationFunctionType.Exp
LN_F = mybir.ActivationFunctionType.Ln
IDENT = mybir.ActivationFunctionType.Identity


# --------------------------------------------------------------------------
# Phase A: seq-sharded projections + local stabilizer
# --------------------------------------------------------------------------
def build_phase_a():
    nc = bacc.Bacc("TRN2", target_bir_lowering=False, debug=False, num_devices=NC)
    xs = nc.dram_tensor("xs", [D_MODEL, 3 * SLA], F32, kind="ExternalInput")
    Wqe = nc.dram_tensor("Wqe", [D_MODEL, D_MODEL], F32, kind="ExternalInput")
    Wke = nc.dram_tensor("Wke", [D_MODEL, D_MODEL], F32, kind="ExternalInput")
    Wv = nc.dram_tensor("Wv", [D_MODEL, D_MODEL], F32, kind="ExternalInput")
    cq = nc.dram_tensor("cq", [128, 4], F32, kind="ExternalInput")
    wmean = nc.dram_tensor("wmean", [128, 1], F32, kind="ExternalInput")
    nh2 = nc.dram_tensor("nh2", [128, 2], F32, kind="ExternalInput")
    ones_r = nc.dram_tensor("ones_r", [1, 128], F32, kind="ExternalInput")
    gqneg = nc.dram_tensor("gqneg", [1, D_MODEL], F32, kind="ExternalInput")
    ident2 = nc.dram_tensor("ident2", [2, 2], F32, kind="ExternalInput")
    misc = nc.dram_tensor("misc", [1, 1], F32, kind="ExternalInput")  # LN_EPS
    kh_out = nc.dram_tensor("kh", [D_MODEL, SLA], F32, kind="ExternalOutput")
    qh_out = nc.dram_tensor("qh", [D_MODEL, SLA], F32, kind="ExternalOutput")
    vhT_out = nc.dram_tensor("vhT", [SLA, D_MODEL], F32, kind="ExternalOutput")
    stab_out = nc.dram_tensor("stab", [1, 1], F32, kind="ExternalOutput")

    with tile.TileContext(nc) as tc:
        with (
            tc.tile_pool(name="wts", bufs=1) as wts,
            tc.tile_pool(name="xin", bufs=1) as xin,
            tc.tile_pool(name="work", bufs=1) as work,
            tc.tile_pool(name="stat", bufs=1) as statp,
            tc.tile_pool(name="outs", bufs=1) as outs,
        ):
            # ---- loads; order chosen so compute can start early:
            # x (q/k/v slices) first -> LN stats chain; then Wk -> kh; Wq; Wv.
            x_r = xin.tile([128, 4, 3 * SLA], F32R)
            nc.gpsimd.dma_start(out=x_r, in_=xs[:, :].rearrange("(c p) f -> p c f", p=128))
            wk_r = wts.tile([128, 4, D_MODEL], F32R)
            nc.gpsimd.dma_start(out=wk_r, in_=Wke[:, :].rearrange("(c p) f -> p c f", p=128))
            wv_r = wts.tile([128, 4, D_MODEL], F32R)
            nc.gpsimd.dma_start(out=wv_r, in_=Wv[:, :].rearrange("(c p) f -> p c f", p=128))
            wq_r = wts.tile([128, 4, D_MODEL], F32R)
            nc.gpsimd.dma_start(out=wq_r, in_=Wqe[:, :].rearrange("(c p) f -> p c f", p=128))
            wm_f = wts.tile([128, 1], F32)
            nc.sync.dma_start(out=wm_f, in_=wmean[:, :])
            wm_r = wts.tile([128, 1], F32R)
            nc.vector.tensor_copy(wm_r, wm_f)
            nh2_f = wts.tile([128, 2], F32)
            nc.sync.dma_start(out=nh2_f, in_=nh2[:, :])
            nh2_r = wts.tile([128, 2], F32R)
            nc.vector.tensor_copy(nh2_r, nh2_f)
            on_f = wts.tile([1, 128], F32)
            nc.sync.dma_start(out=on_f, in_=ones_r[:, :])
            on_r = wts.tile([1, 128], F32R)
            nc.vector.tensor_copy(on_r, on_f)
            gq_f = wts.tile([1, D_MODEL], F32)
            nc.sync.dma_start(out=gq_f, in_=gqneg[:, :])
            gq_r = wts.tile([1, D_MODEL], F32R)
            nc.vector.tensor_copy(gq_r, gq_f)
            id2_f = wts.tile([2, 2], F32)
            nc.sync.dma_start(out=id2_f, in_=ident2[:, :])
            cq_sb = wts.tile([128, 4], F32)
            nc.sync.dma_start(out=cq_sb, in_=cq[:, :])
            misc_sb = wts.tile([1, 1], F32)
            nc.sync.dma_start(out=misc_sb, in_=misc[:, :])

            def q_c(c):
                return x_r[:, c, 0:SLA]

            def k_c(c):
                return x_r[:, c, SLA:2 * SLA]

            def v_c(c):
                return x_r[:, c, 2 * SLA:3 * SLA]

            # ---- LayerNorm stats on q (over d_model, per position).
            # LN is folded into the projection: qh = rstd*(Wq_eff^T q - gq*mu)
            # + cq, so the q-projection itself never waits on this chain.
            mu_r = statp.tile([1, SLA], F32R)
            rsbc_r = work.tile([128, SLA], F32)
            with (
                tc.tile_pool(name="pss", bufs=1, space="PSUM") as pss,
                tc.tile_pool(name="psr", bufs=1, space="PSUM") as psr,
            ):
                mu_ps = pss.tile([1, SLA], F32, tag="mu")
                for c in range(4):
                    nc.tensor.matmul(mu_ps, wm_r, q_c(c), start=(c == 0),
                                     stop=(c == 3), skip_group_check=True)
                qsq_r = work.tile([128, 4, SLA], F32R)
                for c in range(4):
                    nc.vector.tensor_mul(qsq_r[:, c, :], q_c(c), q_c(c))
                msq_ps = pss.tile([1, SLA], F32, tag="msq")
                for c in range(4):
                    nc.tensor.matmul(msq_ps, wm_r, qsq_r[:, c, :], start=(c == 0),
                                     stop=(c == 3), skip_group_check=True)
                nc.scalar.copy(mu_r, mu_ps)
                var_sb = statp.tile([1, SLA], F32)
                nc.vector.tensor_mul(var_sb, mu_r, mu_r)
                nc.vector.tensor_sub(var_sb, msq_ps, var_sb)
                rstd_r = statp.tile([1, SLA], F32R)
                nc.scalar.activation(rstd_r, var_sb, LN_F,
                                     bias=misc_sb[0:1, 0:1], scale=1.0)
                nc.scalar.activation(rstd_r, rstd_r, EXP, bias=0.0, scale=-0.5)
                rsbc_ps = psr.tile([128, SLA], F32, tag="rsbc")
                nc.tensor.matmul(rsbc_ps, on_r, rstd_r, start=True, stop=True,
                                 skip_group_check=True)
                nc.scalar.copy(rsbc_r, rsbc_ps)

            # ---- projections: kh first (only needs Wk), then qh, then vh ----
            kh_sb = outs.tile([128, 4, SLA], F32)
            kh2_r = work.tile([128, 4, SLA], F32R)
            qh_sb = outs.tile([128, 4, SLA], F32)
            vhT_sb = outs.tile([128, 2, D_MODEL], F32)
            with tc.tile_pool(name="psb", bufs=2, space="PSUM") as psb:
                for oc in range(4):
                    kh_ps = psb.tile([128, SLA], F32, tag="kh")
                    for c in range(4):
                        nc.tensor.matmul(kh_ps, wk_r[:, c, ts(oc, 128)],
                                         k_c(c), start=(c == 0),
                                         stop=(c == 3), skip_group_check=True)
                    nc.scalar.copy(kh_sb[:, oc, :], kh_ps)
                    nc.vector.tensor_mul(kh2_r[:, oc, :], kh_sb[:, oc, :],
                                         kh_sb[:, oc, :])
                nc.sync.dma_start(
                    out=kh_out[:, :].rearrange("(c p) f -> p c f", p=128),
                    in_=kh_sb)

                # local stabilizer from kh^2 (small; overlaps Wq/Wv loads)
                hkm = statp.tile([2, 4], F32)
                for oc in range(4):
                    hk_ps = psb.tile([2, SLA], F32, tag="hk", name=f"hk{oc}", bufs=1)
                    nc.tensor.matmul(hk_ps, nh2_r, kh2_r[:, oc, :], start=True,
                                     stop=True, skip_group_check=True)
                    nc.vector.reduce_max(hkm[:, oc:oc + 1], hk_ps,
                                         axis=mybir.AxisListType.X)
                hk2_f = statp.tile([2, 1], F32)
                nc.vector.reduce_max(hk2_f, hkm, axis=mybir.AxisListType.X)
                hkt_ps = psb.tile([1, 2], F32, tag="hkt", bufs=1)
                nc.tensor.transpose(hkt_ps, hk2_f, id2_f)
                stab_sb = statp.tile([1, 1], F32)
                nc.vector.reduce_max(stab_sb, hkt_ps, axis=mybir.AxisListType.X)
                nc.sync.dma_start(out=stab_out[:, :], in_=stab_sb)

                for sc in range(2):
                    vh_ps = psb.tile([128, D_MODEL], F32, tag="vh")
                    for c in range(4):
                        nc.tensor.matmul(vh_ps, v_c(c)[:, ts(sc, 128)],
                                         wv_r[:, c, :], start=(c == 0),
                                         stop=(c == 3), skip_group_check=True)
                    nc.scalar.copy(vhT_sb[:, sc, :], vh_ps)
                nc.sync.dma_start(
                    out=vhT_out[:, :].rearrange("(s p) f -> p s f", p=128),
                    in_=vhT_sb)

                for oc in range(4):
                    qh_ps = psb.tile([128, SLA], F32, tag="qh")
                    for c in range(4):
                        nc.tensor.matmul(qh_ps, wq_r[:, c, ts(oc, 128)],
                                         q_c(c), start=(c == 0),
                                         stop=False, skip_group_check=True)
                    nc.tensor.matmul(qh_ps, gq_r[0:1, ts(oc, 128)], mu_r,
                                     start=False, stop=True,
                                     skip_group_check=True)
                    nc.vector.tensor_mul(qh_sb[:, oc, :], qh_ps, rsbc_r)
                    nc.scalar.activation(qh_sb[:, oc, :], qh_sb[:, oc, :], IDENT,
                                         bias=cq_sb[:, oc:oc + 1], scale=1.0)
                nc.sync.dma_start(
                    out=qh_out[:, :].rearrange("(c p) f -> p c f", p=128),
                    in_=qh_sb)
    nc.compile()
    return nc


# --------------------------------------------------------------------------
# Phase B: head-sharded Performer attention + FC row-block
# --------------------------------------------------------------------------
def build_phase_b(debug=False):
    nc = bacc.Bacc("TRN2", target_bir_lowering=False, debug=False, num_devices=NC)
    khh = nc.dram_tensor("khh", [D_K, N], F32, kind="ExternalInput")
    qhh = nc.dram_tensor("qhh", [D_K, N], F32, kind="ExternalInput")
    vht = nc.dram_tensor("vht", [128, NCH * D_V], F32, kind="ExternalInput")
    rft = nc.dram_tensor("rft", [D_K, M], F32, kind="ExternalInput")
    rneg = nc.dram_tensor("rneg", [D_K, M], F32, kind="ExternalInput")
    wfc = nc.dram_tensor("wfc", [D_V, D_MODEL], F32, kind="ExternalInput")
    pairmask = nc.dram_tensor("pairmask", [C, 2 * C], F32, kind="ExternalInput")
    identm = nc.dram_tensor("identm", [128, 128], F32, kind="ExternalInput")
    onescol = nc.dram_tensor("onescol", [128, 1], F32, kind="ExternalInput")
    stabcol = nc.dram_tensor("stabcol", [128, 1], F32, kind="ExternalInput")
    c2 = nc.dram_tensor("c2", [128, 2 * NCH], F32, kind="ExternalInput")
    epsk = nc.dram_tensor("epsk", [128, NCH], F32, kind="ExternalInput")
    eps_row = nc.dram_tensor("eps_row", [1, N], F32, kind="ExternalInput")
    zeros66 = nc.dram_tensor("zeros66", [128, 3 * 66], F32, kind="ExternalInput")
    misc = nc.dram_tensor("misc", [1, 1], F32, kind="ExternalInput")  # M*eps
    out_d = nc.dram_tensor("out", [N, D_MODEL], F32, kind="ExternalOutput")

    with tile.TileContext(nc) as tc:
        with (
            tc.tile_pool(name="consts", bufs=1) as consts,
            tc.tile_pool(name="krows", bufs=1) as krows,
            tc.tile_pool(name="feat", bufs=1) as feat,
            tc.tile_pool(name="ktrp", bufs=1) as ktrp,
            tc.tile_pool(name="ktT", bufs=NCH) as ktTp,
            tc.tile_pool(name="atp", bufs=2) as atp,
            tc.tile_pool(name="ssb", bufs=NPAIR + 1) as ssbp,
            tc.tile_pool(name="post", bufs=4) as post,
            tc.tile_pool(name="outp", bufs=3) as outp,
        ):
            # ---- casting loads ----
            qhr = krows.tile([D_K, N], F32R)
            nc.gpsimd.dma_start(out=qhr, in_=qhh[:, :])
            khr = krows.tile([D_K, N], F32R)
            nc.gpsimd.dma_start(out=khr, in_=khh[:, :])
            vha = krows.tile([128, NCH, 66], F32R)
            nc.gpsimd.dma_start(
                out=vha[:, :, 0:D_V],
                in_=vht[:, :].rearrange("p (ch f) -> p ch f", ch=NCH))

            # ---- plain loads + engine conversions for small consts ----
            rft_f = consts.tile([D_K, M], F32)
            nc.sync.dma_start(out=rft_f, in_=rft[:, :])
            rft_r = consts.tile([D_K, M], F32R)
            nc.vector.tensor_copy(rft_r, rft_f)
            rneg_f = consts.tile([D_K, M], F32)
            nc.sync.dma_start(out=rneg_f, in_=rneg[:, :])
            rneg_r = consts.tile([D_K, M], F32R)
            nc.gpsimd.tensor_copy(rneg_r, rneg_f)
            wfc_f = consts.tile([D_V, D_MODEL], F32)
            nc.sync.dma_start(out=wfc_f, in_=wfc[:, :])
            wfc_r = consts.tile([D_V, D_MODEL], F32R)
            nc.vector.tensor_copy(wfc_r, wfc_f)
            id_f = consts.tile([128, 128], F32)
            nc.sync.dma_start(out=id_f, in_=identm[:, :])
            id_r = consts.tile([128, 128], F32R)
            nc.gpsimd.tensor_copy(id_r, id_f)
            onc_f = consts.tile([128, 1], F32)
            nc.sync.dma_start(out=onc_f, in_=onescol[:, :])
            onc_r = consts.tile([128, 1], F32R)
            nc.vector.tensor_copy(onc_r, onc_f)
            pmask_sb = consts.tile([C, 2 * C], F32)
            nc.sync.dma_start(out=pmask_sb, in_=pairmask[:, :])
            stab_sb = consts.tile([128, 1], F32)
            nc.sync.dma_start(out=stab_sb, in_=stabcol[:, :])
            c2_sb = consts.tile([128, 2 * NCH], F32)
            nc.sync.dma_start(out=c2_sb, in_=c2[:, :])
            epsk_sb = consts.tile([128, NCH], F32)
            nc.sync.dma_start(out=epsk_sb, in_=epsk[:, :])
            z66_sb = consts.tile([128, 3, 66], F32)
            nc.sync.dma_start(
                out=z66_sb, in_=zeros66[:, :].rearrange("p (a b) -> p a b", a=3))
            misc_sb = consts.tile([1, 1], F32)
            nc.sync.dma_start(out=misc_sb, in_=misc[:, :])

            # vha constant columns: 64 -> 1.0, 65 -> 0.0
            nc.gpsimd.tensor_copy(
                vha[:, :, D_V:D_V + 2],
                c2_sb[:, :].rearrange("p (ch f) -> p ch f", ch=NCH))

            # kh^2 rows (squares of the casting-DMA'd kh)
            kh2r = krows.tile([D_K, N], F32R)
            for s in range(NSL):
                nc.vector.tensor_mul(kh2r[:, ts(s, SL)], khr[:, ts(s, SL)],
                                     khr[:, ts(s, SL)])

            # ---- q~ features [m-major: 128 x 3 x N] ----
            qt_feat = feat.tile([128, 3, N], F32R)
            nc.gpsimd.dma_start(out=qt_feat[11:12, 2, :], in_=eps_row[:, :])
            with (
                tc.tile_pool(name="psqp", bufs=2, space="PSUM") as psqp,
                tc.tile_pool(name="pssp", bufs=2, space="PSUM") as pssp,
                tc.tile_pool(name="qtmp2", bufs=2) as qtmp2,
            ):
                for half in range(2):       # two 1024-wide exp batches per mc
                    for mc in range(3):
                        mrows = 128 if mc < 2 else 10
                        qp_ps = psqp.tile([128, 2 * SL], F32, tag="qp")
                        for sub in range(2):
                            s = 2 * half + sub
                            nc.tensor.matmul(
                                qp_ps[0:mrows, ts(sub, SL)],
                                rft_r[:, ds(mc * 128, mrows)],
                                qhr[:, ts(s, SL)], start=True, stop=True,
                                skip_group_check=True)
                        nc.scalar.activation(
                            qt_feat[0:mrows, mc, ts(half, 2 * SL)],
                            qp_ps[0:mrows, :], EXP, bias=0.0, scale=1.0)
                for s in range(NSL):
                    sp_ps = pssp.tile([1, SL], F32, tag="sp")
                    nc.tensor.matmul(sp_ps, onc_r, qt_feat[:, 0, ts(s, SL)],
                                     start=True, stop=False, skip_group_check=True)
                    nc.tensor.matmul(sp_ps, onc_r, qt_feat[:, 1, ts(s, SL)],
                                     start=False, stop=False, skip_group_check=True)
                    nc.tensor.matmul(sp_ps, onc_r[0:10, :],
                                     qt_feat[0:10, 2, ts(s, SL)],
                                     start=False, stop=True, skip_group_check=True)
                    sp_sb = qtmp2.tile([1, SL], F32, tag="sp_sb")
                    nc.scalar.activation(sp_sb, sp_ps, IDENT,
                                         bias=misc_sb[0:1, 0:1], scale=1.0)
                    nc.gpsimd.dma_start(out=qt_feat[10:11, 2, ts(s, SL)],
                                        in_=sp_sb)

            # ---- stage 1: k~ features, z accumulation, transposes,
            #      then (per pair) attention blocks + state scan ----
            ktr = ktrp.tile([128, NCH, 268], F32R)
            nc.vector.tensor_copy(
                ktr[:, :, 266:267],
                epsk_sb[:, :].rearrange("p (ch f) -> p ch f", ch=NCH))

            s_tiles = [ssbp.tile([128, 3, 66], F32R, tag="ssb", name=f"ssb{i}")
                       for i in range(NPAIR + 1)]
            nc.vector.tensor_copy(s_tiles[0][:, 0:2, :], z66_sb[:, 0:2, :])
            nc.scalar.copy(s_tiles[0][0:12, 2:3, :], z66_sb[0:12, 2:3, :])

            ktT_tiles = {}
            at1_list = []
            at2_list = []
            zcol2 = feat.tile([128, 3, 2], F32R)
            nc.vector.tensor_copy(zcol2[:, :, 1:2], z66_sb[:, :, 0:1])
            with (
                tc.tile_pool(name="pskp", bufs=2, space="PSUM") as pskp,
                tc.tile_pool(name="pstr", bufs=1, space="PSUM") as pstr,
                tc.tile_pool(name="psat", bufs=2, space="PSUM") as psat,
                tc.tile_pool(name="pssd", bufs=2, space="PSUM") as pssd,
            ):
                for ch in range(NCH):
                    # k features for chunk ch
                    kp_ps = pskp.tile([C, M], F32, tag="kp")
                    nc.tensor.matmul(kp_ps, khr[:, ts(ch, C)], rft_r,
                                     start=True, stop=False, skip_group_check=True)
                    nc.tensor.matmul(kp_ps, kh2r[:, ts(ch, C)], rneg_r,
                                     start=False, stop=True, skip_group_check=True)
                    with nc.allow_low_precision(reason="fp32r accum ~ fp32"):
                        nc.scalar.activation(
                            ktr[:, ch, 0:M], kp_ps, EXP,
                            bias=stab_sb[:, 0:1], scale=1.0,
                            accum_out=ktr[:, ch, 267:268])
                    # transpose the chunk's features to m-major
                    tp_ps = pstr.tile([128, 3, 128], F32R, tag="tp")
                    nc.tensor.transpose(tp_ps[:, 0, :], ktr[:, ch, 0:128], id_r)
                    nc.tensor.transpose(tp_ps[:, 1, :], ktr[:, ch, 128:256], id_r)
                    nc.tensor.transpose(tp_ps[0:12, 2, :], ktr[:, ch, 256:268], id_r)
                    ktT = ktTp.tile([128, 3, C], F32R, tag="ktT", name=f"ktT{ch}")
                    if ch % 2 == 0:
                        nc.vector.tensor_copy(ktT[:, 0:2, :], tp_ps[:, 0:2, :])
                        nc.scalar.copy(ktT[0:12, 2:3, :], tp_ps[0:12, 2:3, :])
                    else:
                        nc.scalar.copy(ktT[:, 0:2, :], tp_ps[:, 0:2, :])
                        nc.vector.tensor_copy(ktT[0:12, 2:3, :], tp_ps[0:12, 2:3, :])
                    ktT_tiles[ch] = ktT

                    if ch % 2 == 1:
                        i = ch // 2
                        a, b = 2 * i, 2 * i + 1
                        # pair attention block: keys a x queries {a, b}
                        at1_ps = psat.tile([C, 2 * C], F32, tag="at1")
                        ka = ktT_tiles[a]
                        for mc in range(2):
                            nc.tensor.matmul(at1_ps, ka[:, mc, :],
                                             qt_feat[:, mc, ts(i, 2 * C)],
                                             start=(mc == 0), stop=False,
                                             skip_group_check=True)
                        nc.tensor.matmul(at1_ps, ka[0:12, 2, :],
                                         qt_feat[0:12, 2, ts(i, 2 * C)],
                                         start=False, stop=True,
                                         skip_group_check=True)
                        at1_r = atp.tile([C, 2 * C], F32R, tag="at1r",
                                         name=f"at1r{i}", bufs=NPAIR)
                        nc.vector.tensor_mul(at1_r, at1_ps, pmask_sb)
                        at1_list.append(at1_r)
                        # odd diag block: keys b x queries b
                        at2_ps = psat.tile([C, C], F32, tag="at2", bufs=1)
                        kb = ktT_tiles[b]
                        for mc in range(2):
                            nc.tensor.matmul(at2_ps, kb[:, mc, :],
                                             qt_feat[:, mc, ts(b, C)],
                                             start=(mc == 0), stop=False,
                                             skip_group_check=True)
                        nc.tensor.matmul(at2_ps, kb[0:12, 2, :],
                                         qt_feat[0:12, 2, ts(b, C)],
                                         start=False, stop=True,
                                         skip_group_check=True)
                        at2_r = atp.tile([C, C], F32R, tag="at2r",
                                         name=f"at2r{i}", bufs=NPAIR)
                        nc.vector.tensor_mul(at2_r, at2_ps, pmask_sb[:, 0:C])
                        at2_list.append(at2_r)

                        # state update for the pair
                        sd_ps = pssd.tile([128, 3, 66], F32, tag="sd")
                        for mc, cols in ((0, (0, 128)), (1, (128, 256)),
                                         (2, (256, 268))):
                            dst = sd_ps[:, mc, :] if mc < 2 else sd_ps[0:12, 2, :]
                            for j, ch2 in enumerate((a, b)):
                                nc.tensor.matmul(dst, ktr[:, ch2, cols[0]:cols[1]],
                                                 vha[:, ch2, 0:66], start=(j == 0),
                                                 stop=(j == 1),
                                                 skip_group_check=True)
                        nc.vector.tensor_add(s_tiles[i + 1][:, 0:2, 0:66],
                                             s_tiles[i][:, 0:2, 0:66],
                                             sd_ps[:, 0:2, :])
                        nc.vector.tensor_add(s_tiles[i + 1][0:12, 2:3, 0:66],
                                             s_tiles[i][0:12, 2:3, 0:66],
                                             sd_ps[0:12, 2:3, :])

            # z column = final state's column 64 (already column-major)
            zfin = s_tiles[NPAIR]
            nc.vector.tensor_copy(zcol2[:, 0:2, 0:1], zfin[:, 0:2, 64:65])
            nc.scalar.copy(zcol2[0:12, 2:3, 0:1], zfin[0:12, 2:3, 64:65])

            # ---- per-chunk output, normalize, FC ----
            with (
                tc.tile_pool(name="pso", bufs=3, space="PSUM") as pso,
                tc.tile_pool(name="psfc", bufs=2, space="PSUM") as psfc,
                tc.tile_pool(name="pstr2", bufs=2, space="PSUM") as pstr2,
            ):
                for ch in range(NCH):
                    i = ch // 2
                    o_ps = pso.tile([C, 68], F32, tag="o")
                    for mc in range(2):
                        nc.tensor.matmul(o_ps[:, 0:66], qt_feat[:, mc, ts(ch, C)],
                                         s_tiles[i][:, mc, 0:66],
                                         start=(mc == 0), stop=False,
                                         skip_group_check=True)
                    nc.tensor.matmul(o_ps[:, 0:66], qt_feat[0:12, 2, ts(ch, C)],
                                     s_tiles[i][0:12, 2, 0:66],
                                     start=False, stop=False, skip_group_check=True)
                    if ch % 2 == 0:
                        nc.tensor.matmul(
                            o_ps[:, 0:66], at1_list[i][:, 0:C],
                            vha[:, ch, :], start=False, stop=True,
                            skip_group_check=True)
                    else:
                        nc.tensor.matmul(
                            o_ps[:, 0:66], at1_list[i][:, C:2 * C],
                            vha[:, ch - 1, :], start=False, stop=False,
                            skip_group_check=True)
                        nc.tensor.matmul(
                            o_ps[:, 0:66], at2_list[i],
                            vha[:, ch, :], start=False, stop=True,
                            skip_group_check=True)
                    # d = q~ . z in columns 66:68 (col 67 is a zero pad)
                    for mc in range(2):
                        nc.tensor.matmul(o_ps[:, 66:68], qt_feat[:, mc, ts(ch, C)],
                                         zcol2[:, mc, :], start=(mc == 0),
                                         stop=False, skip_group_check=True)
                    nc.tensor.matmul(o_ps[:, 66:68], qt_feat[0:12, 2, ts(ch, C)],
                                     zcol2[0:12, 2, :], start=False, stop=True,
                                     skip_group_check=True)
                    dcols = post.tile([C, 2, 2], F32, tag="dcols")
                    nc.vector.tensor_copy(
                        dcols, o_ps[:, 64:68].rearrange("p (a b) -> p a b", a=2))
                    dd = post.tile([C, 1], F32, tag="dd")
                    nc.vector.tensor_mul(dd, dcols[:, 0, 0:1], dcols[:, 1, 0:1])
                    rec = post.tile([C, 1], F32, tag="rec")
                    nc.vector.reciprocal(rec, dd)
                    attn_r = post.tile([C, D_V], F32R, tag="attn")
                    nc.vector.tensor_scalar_mul(attn_r, o_ps[:, 0:D_V], rec)
                    tr_ps = pstr2.tile([D_V, C], F32R, tag="tr")
                    nc.tensor.transpose(tr_ps, attn_r, id_r)
                    attnT_r = post.tile([D_V, C], F32R, tag="attnT")
                    if ch % 2 == 0:
                        nc.scalar.copy(attnT_r, tr_ps)
                    else:
                        nc.vector.tensor_copy(attnT_r, tr_ps)
                    fc_ps = psfc.tile([C, D_MODEL], F32, tag="fc")
                    nc.tensor.matmul(fc_ps, attnT_r, wfc_r, start=True, stop=True,
                                     skip_group_check=True)
                    o_sb = outp.tile([C, D_MODEL], F32, tag="osb")
                    if ch % 2 == 0:
                        nc.vector.tensor_copy(o_sb, fc_ps)
                    else:
                        nc.scalar.copy(o_sb, fc_ps)
                    nc.sync.dma_start(out=out_d[ts(ch, C), :], in_=o_sb)
    nc.compile()
    return nc


# --------------------------------------------------------------------------
# Host orchestration
# --------------------------------------------------------------------------
_CACHE = {}


def _get_programs():
    if "a" not in _CACHE:
        _CACHE["a"] = build_phase_a()
        _CACHE["b"] = build_phase_b()
    return _CACHE["a"], _CACHE["b"]


def _prep_a_maps(q, k, v, Wq, Wk, Wv, gamma, beta):
    qT = np.ascontiguousarray(q[0].T)
    kT = np.ascontiguousarray(k[0].T)
    vT = np.ascontiguousarray(v[0].T)
    Wqe = np.ascontiguousarray((gamma[:, None] * Wq) * SCALE)
    Wke = np.ascontiguousarray(Wk * SCALE)
    cq_all = (beta @ Wq) * SCALE                       # [512]
    cq = np.ascontiguousarray(cq_all.reshape(4, 128).T)
    wmean = np.full((128, 1), 1.0 / D_MODEL, np.float32)
    nh2 = np.zeros((128, 2), np.float32)
    nh2[0:64, 0] = -0.5
    nh2[64:128, 1] = -0.5
    ones_r = np.ones((1, 128), np.float32)
    gqneg = np.ascontiguousarray(-Wqe.sum(axis=0)[None, :])
    ident2 = np.eye(2, dtype=np.float32)
    misc = np.full((1, 1), LN_EPS, np.float32)
    in_a = []
    for j in range(NC):
        sl = slice(j * SLA, (j + 1) * SLA)
        xs = np.concatenate([qT[:, sl], kT[:, sl], vT[:, sl]], axis=1)
        in_a.append({
            "xs": np.ascontiguousarray(xs),
            "Wqe": Wqe, "Wke": Wke, "Wv": np.ascontiguousarray(Wv),
            "cq": cq, "wmean": wmean, "nh2": nh2, "ones_r": ones_r,
            "gqneg": gqneg, "ident2": ident2, "misc": misc,
        })
    return in_a


def _prep_b_maps(W_fc, rf, res_a):
    kh_full = np.concatenate([r["kh"] for r in res_a], axis=1)   # [512, N]
    qh_full = np.concatenate([r["qh"] for r in res_a], axis=1)   # [512, N]
    vh_full = np.concatenate([r["vhT"] for r in res_a], axis=0)  # [N, 512]
    k_stab = np.float32(max(r["stab"][0, 0] for r in res_a))

    rftT = np.ascontiguousarray(rf.T)                  # [64, 266]
    rneg = np.full((D_K, M), -0.5, np.float32)
    tri = np.triu(np.ones((C, C), np.float32))
    pairmask = np.ascontiguousarray(
        np.concatenate([tri, np.ones((C, C), np.float32)], axis=1))
    identm = np.eye(128, dtype=np.float32)
    onescol = np.ones((128, 1), np.float32)
    stabcol = np.full((128, 1), -k_stab, np.float32)
    c2 = np.zeros((128, 2 * NCH), np.float32)
    c2[:, 0::2] = 1.0                                  # vha col 64 = 1, col 65 = 0
    epsk = np.full((128, NCH), KERNEL_EPS, np.float32)
    eps_row = np.full((1, N), KERNEL_EPS, np.float32)
    zeros66 = np.zeros((128, 3 * 66), np.float32)
    misc = np.full((1, 1), M * KERNEL_EPS, np.float32)

    in_b = []
    for h in range(NC):
        rows = slice(h * D_K, (h + 1) * D_K)
        vh_h = vh_full[:, h * D_V:(h + 1) * D_V]       # [N, 64]
        vht = np.ascontiguousarray(
            vh_h.reshape(NCH, 128, D_V).transpose(1, 0, 2).reshape(128, NCH * D_V))
        in_b.append({
            "khh": np.ascontiguousarray(kh_full[rows]),
            "qhh": np.ascontiguousarray(qh_full[rows]),
            "vht": vht,
            "rft": rftT, "rneg": rneg,
            "wfc": np.ascontiguousarray(W_fc[rows, :] * float(M)),
            "pairmask": pairmask, "identm": identm, "onescol": onescol,
            "stabcol": stabcol, "c2": c2, "epsk": epsk, "eps_row": eps_row,
            "zeros66": zeros66, "misc": misc,
        })
    return in_b


def _cast_all(*arrs):
    return [np.asarray(a, np.float32) for a in arrs]


def kernel(q, k, v, Wq, Wk, Wv, W_fc, b_fc, gamma, beta, rf):
    q, k, v, Wq, Wk, Wv, W_fc, b_fc, gamma, beta, rf = _cast_all(
        q, k, v, Wq, Wk, Wv, W_fc, b_fc, gamma, beta, rf)

    nc_a, nc_b = _get_programs()
    cores = list(range(NC))

    in_a = _prep_a_maps(q, k, v, Wq, Wk, Wv, gamma, beta)
    res_a = run_bass_kernel_spmd(nc_a, in_a, core_ids=cores)

    in_b = _prep_b_maps(W_fc, rf, res_a.results)
    res_b = run_bass_kernel_spmd(nc_b, in_b, core_ids=cores)

    out = np.zeros((N, D_MODEL), np.float32)
    for r in res_b.results:
        out += r["out"]
    out += b_fc[None, :]
    out += q[0]
    return out[None].astype(np.float32)


def trace_args(inputs):
    """For test.py: returns [(phase, nc, in_maps), ...] re-runnable with trace."""
    q, k, v, Wq, Wk, Wv, W_fc, b_fc, gamma, beta, rf = _cast_all(
        inputs["q"], inputs["k"], inputs["v"], inputs["Wq"], inputs["Wk"],
        inputs["Wv"], inputs["W_fc"], inputs["b_fc"], inputs["gamma"],
        inputs["beta"], inputs["rf"])
    nc_a, nc_b = _get_programs()
    in_a = _prep_a_maps(q, k, v, Wq, Wk, Wv, gamma, beta)
    res_a = run_bass_kernel_spmd(nc_a, in_a, core_ids=list(range(NC)))
    in_b = _prep_b_maps(W_fc, rf, res_a.results)
    return [("a", nc_a, in_a), ("b", nc_b, in_b)]


if __name__ == "__main__":
    rng = np.random.default_rng(0)
    inputs = {
        "q": rng.standard_normal((1, N, D_MODEL)).astype(np.float32),
        "k": rng.standard_normal((1, N, D_MODEL)).astype(np.float32),
        "v": rng.standard_normal((1, N, D_MODEL)).astype(np.float32),
        "Wq": (rng.standard_normal((D_MODEL, 512)) * 0.04).astype(np.float32),
        "Wk": (rng.standard_normal((D_MODEL, 512)) * 0.04).astype(np.float32),
        "Wv": (rng.standard_normal((D_MODEL, 512)) * 0.04).astype(np.float32),
        "W_fc": (rng.standard_normal((512, D_MODEL)) * 0.04).astype(np.float32),
        "b_fc": np.zeros(D_MODEL, np.float32),
        "gamma": np.ones(D_MODEL, np.float32),
        "beta": np.zeros(D_MODEL, np.float32),
        "rf": rng.standard_normal((M, D_K)).astype(np.float32),
    }
    out = kernel(**inputs)
    print("kernel output", out.shape, out.dtype)


# revision 28
# speedup vs baseline: 1.0572x; 1.0572x over previous
"""Performer attention (causal, kernelized) — Trainium2 Bass kernel, v3.

Two launches on 8 cores:

  A) seq-sharded prep: core j owns 256 sequence positions and computes, for
     ALL 8 heads at once: kh (scaled k-projection), the LayerNorm-folded and
     scaled/biased q-projection qh, the v-projection in seq-major layout,
     and the local stabilizer max(h_k).  Each (position, head) projection is
     computed exactly once fleet-wide.

  B) head-sharded attention: core h owns head h end-to-end: Performer
     feature maps, the causal chunked prefix scan, output normalization and
     its row-block of the FC (W_fc row-sharded; host sums partials and adds
     bias + residual).  The exact global k_stab (host max over the 8 phase-A
     stabs) is folded into the k-feature exp bias — no approximation.

All big matmuls use float32r (4x PE throughput at free >= 256, ~2e-4
relative error; end-to-end max-rel stays ~1e-3).  Algebra notes (validated
against the reference):
  - q LayerNorm folded: Wq_eff = diag(gamma) Wq * scale, bias cq = beta@Wq*scale,
    applied to (q - mu) * rstd with rstd = exp(-0.5 ln(var + eps)).
  - exp(h_q + (proj_q - h_q)) == exp(proj_q): q-side stabilizer cancels.
  - k feature: exp(proj_k + h_k - k_stab) via the augmented contraction
    [kh; kh^2] . [rf^T; -0.5] plus a per-partition bias of -k_stab in the exp.
  - +KERNEL_EPS becomes extra features: q~ rows 266/267 = (sum_m exp_q + m*eps,
    eps); k~ cols 266/267 = (eps, sum_m exp_k); the global 1/sqrt(m) cancels
    except 1/c^2 folded into W_fc.
  - causal prefix scan chunked at C=128 with states per chunk-PAIR: the
    in-pair cross term (keys of even chunk x queries of odd chunk) rides in a
    [128 x 256] pair-attention block at full fp32r speed.
  - the non-causal normalizer d = q~ . z (z = column sums of k~) is computed
    from a separate z accumulation: o columns 64/66 carry D and d; no state
    column fixups needed.
  - the reference's |d|<=1e-6 guard is dead for any realistic data (d ~ 1e3+)
    and is omitted.
"""

import sys
for _p in ("/opt/trn_rl_repo", "/root/.axon_site/_ro/trn_rl_repo"):
    if _p not in sys.path:
        sys.path.append(_p)

import numpy as np

import concourse.bass as bass
from concourse import bacc
import concourse.mybir as mybir
import concourse.tile as tile
from concourse.bass import ts, ds
from concourse.bass_utils import run_bass_kernel_spmd

F32 = mybir.dt.float32
F32R = mybir.dt.float32r
NC = 8
N = 2048
D_MODEL = 512
D_K = 64
D_V = 64
M = 266
C = 128
NCH = N // C            # 16 chunks
NPAIR = NCH // 2        # 8 chunk pairs
SLA = N // NC           # 256 seq positions per phase-A core
NSL = 4                 # 512-wide slices of the full sequence
SL = 512
KERNEL_EPS = 1e-4
LN_EPS = 1e-6
SCALE = float(D_MODEL) ** (-0.25)
EXP = mybir.ActivationFunctionType.Exp
LN_F = mybir.ActivationFunctionType.Ln
IDENT = mybir.ActivationFunctionType.Identity


# --------------------------------------------------------------------------
# Phase A: seq-sharded projections + local stabilizer
# --------------------------------------------------------------------------
def build_phase_a():
    nc = bacc.Bacc("TRN2", target_bir_lowering=False, debug=False, num_devices=NC)
    xs = nc.dram_tensor("xs", [D_MODEL, 3 * SLA], F32, kind="ExternalInput")
    Wqe = nc.dram_tensor("Wqe", [D_MODEL, D_MODEL], F32, kind="ExternalInput")
    Wke = nc.dram_tensor("Wke", [D_MODEL, D_MODEL], F32, kind="ExternalInput")
    Wv = nc.dram_tensor("Wv", [D_MODEL, D_MODEL], F32, kind="ExternalInput")
    cq = nc.dram_tensor("cq", [128, 4], F32, kind="ExternalInput")
    wmean = nc.dram_tensor("wmean", [128, 1], F32, kind="ExternalInput")
    nh2 = nc.dram_tensor("nh2", [128, 2], F32, kind="ExternalInput")
    ones_r = nc.dram_tensor("ones_r", [1, 128], F32, kind="ExternalInput")
    gqneg = nc.dram_tensor("gqneg", [1, D_MODEL], F32, kind="ExternalInput")
    ident2 = nc.dram_tensor("ident2", [2, 2], F32, kind="ExternalInput")
    misc = nc.dram_tensor("misc", [1, 1], F32, kind="ExternalInput")  # LN_EPS
    kh_out = nc.dram_tensor("kh", [D_MODEL, SLA], F32, kind="ExternalOutput")
    qh_out = nc.dram_tensor("qh", [D_MODEL, SLA], F32, kind="ExternalOutput")
    vhT_out = nc.dram_tensor("vhT", [SLA, D_MODEL], F32, kind="ExternalOutput")
    stab_out = nc.dram_tensor("stab", [1, 1], F32, kind="ExternalOutput")

    with tile.TileContext(nc) as tc:
        with (
            tc.tile_pool(name="wts", bufs=1) as wts,
            tc.tile_pool(name="xin", bufs=1) as xin,
            tc.tile_pool(name="work", bufs=1) as work,
            tc.tile_pool(name="stat", bufs=1) as statp,
            tc.tile_pool(name="outs", bufs=1) as outs,
        ):
            # ---- loads; order chosen so compute can start early:
            # x (q/k/v slices) first -> LN stats chain; then Wk -> kh; Wq; Wv.
            q_r = xin.tile([128, 4, SLA], F32R)
            nc.gpsimd.dma_start(out=q_r, in_=xs[:, 0:SLA].rearrange("(c p) f -> p c f", p=128))
            k_r = xin.tile([128, 4, SLA], F32R)
            nc.gpsimd.dma_start(out=k_r, in_=xs[:, SLA:2 * SLA].rearrange("(c p) f -> p c f", p=128))
            v_r = xin.tile([128, 4, SLA], F32R)
            nc.gpsimd.dma_start(out=v_r, in_=xs[:, 2 * SLA:3 * SLA].rearrange("(c p) f -> p c f", p=128))
            wk_r = wts.tile([128, 4, D_MODEL], F32R)
            nc.gpsimd.dma_start(out=wk_r, in_=Wke[:, :].rearrange("(c p) f -> p c f", p=128))
            wv_r = wts.tile([128, 4, D_MODEL], F32R)
            nc.gpsimd.dma_start(out=wv_r, in_=Wv[:, :].rearrange("(c p) f -> p c f", p=128))
            wq_r = wts.tile([128, 4, D_MODEL], F32R)
            nc.gpsimd.dma_start(out=wq_r, in_=Wqe[:, :].rearrange("(c p) f -> p c f", p=128))
            wm_f = wts.tile([128, 1], F32)
            nc.sync.dma_start(out=wm_f, in_=wmean[:, :])
            wm_r = wts.tile([128, 1], F32R)
            nc.vector.tensor_copy(wm_r, wm_f)
            nh2_f = wts.tile([128, 2], F32)
            nc.sync.dma_start(out=nh2_f, in_=nh2[:, :])
            nh2_r = wts.tile([128, 2], F32R)
            nc.vector.tensor_copy(nh2_r, nh2_f)
            on_f = wts.tile([1, 128], F32)
            nc.sync.dma_start(out=on_f, in_=ones_r[:, :])
            on_r = wts.tile([1, 128], F32R)
            nc.vector.tensor_copy(on_r, on_f)
            gq_f = wts.tile([1, D_MODEL], F32)
            nc.sync.dma_start(out=gq_f, in_=gqneg[:, :])
            gq_r = wts.tile([1, D_MODEL], F32R)
            nc.vector.tensor_copy(gq_r, gq_f)
            id2_f = wts.tile([2, 2], F32)
            nc.sync.dma_start(out=id2_f, in_=ident2[:, :])
            cq_sb = wts.tile([128, 4], F32)
            nc.sync.dma_start(out=cq_sb, in_=cq[:, :])
            misc_sb = wts.tile([1, 1], F32)
            nc.sync.dma_start(out=misc_sb, in_=misc[:, :])
            warm = statp.tile([1, 1], F32)
            nc.scalar.activation(warm, misc_sb, LN_F, bias=1.0, scale=1.0)
            nc.scalar.activation(warm, warm, EXP, bias=0.0, scale=0.0)

            def q_c(c):
                return q_r[:, c, :]

            def k_c(c):
                return k_r[:, c, :]

            def v_c(c):
                return v_r[:, c, :]

            # ---- LayerNorm stats on q (over d_model, per position).
            # LN is folded into the projection: qh = rstd*(Wq_eff^T q - gq*mu)
            # + cq, so the q-projection itself never waits on this chain.
            mu_r = statp.tile([1, SLA], F32R)
            rsbc_r = work.tile([128, SLA], F32)
            with (
                tc.tile_pool(name="pss", bufs=1, space="PSUM") as pss,
                tc.tile_pool(name="psr", bufs=1, space="PSUM") as psr,
            ):
                mu_ps = pss.tile([1, SLA], F32, tag="mu")
                for c in range(4):
                    nc.tensor.matmul(mu_ps, wm_r, q_c(c), start=(c == 0),
                                     stop=(c == 3), skip_group_check=True)
                qsq_r = work.tile([128, 4, SLA], F32R)
                for c in range(4):
                    nc.vector.tensor_mul(qsq_r[:, c, :], q_c(c), q_c(c))
                msq_ps = pss.tile([1, SLA], F32, tag="msq")
                for c in range(4):
                    nc.tensor.matmul(msq_ps, wm_r, qsq_r[:, c, :], start=(c == 0),
                                     stop=(c == 3), skip_group_check=True)
                nc.vector.tensor_copy(mu_r, mu_ps)
                var_sb = statp.tile([1, SLA], F32)
                nc.vector.tensor_mul(var_sb, mu_r, mu_r)
                nc.vector.tensor_sub(var_sb, msq_ps, var_sb)
                rstd_r = statp.tile([1, SLA], F32R)
                nc.scalar.activation(rstd_r, var_sb, LN_F,
                                     bias=misc_sb[0:1, 0:1], scale=1.0)
                nc.scalar.activation(rstd_r, rstd_r, EXP, bias=0.0, scale=-0.5)

            # ---- projections: kh first (only needs Wk), then qh, then vh ----
            kh_sb = outs.tile([128, 4, SLA], F32)
            kh2_r = work.tile([128, 4, SLA], F32R)
            qh_sb = outs.tile([128, 4, SLA], F32)
            vhT_sb = outs.tile([128, 2, D_MODEL], F32)
            with tc.tile_pool(name="psb", bufs=2, space="PSUM") as psb:
                for oc in range(4):
                    kh_ps = psb.tile([128, SLA], F32, tag="kh")
                    for c in range(4):
                        nc.tensor.matmul(kh_ps, wk_r[:, c, ts(oc, 128)],
                                         k_c(c), start=(c == 0),
                                         stop=(c == 3), skip_group_check=True)
                    nc.scalar.copy(kh_sb[:, oc, :], kh_ps)
                    nc.vector.tensor_mul(kh2_r[:, oc, :], kh_sb[:, oc, :],
                                         kh_sb[:, oc, :])
                nc.sync.dma_start(
                    out=kh_out[:, :].rearrange("(c p) f -> p c f", p=128),
                    in_=kh_sb)

                # local stabilizer from kh^2 (small; overlaps Wq/Wv loads)
                hkm = statp.tile([2, 4], F32)
                for oc in range(4):
                    hk_ps = psb.tile([2, SLA], F32, tag="hk", name=f"hk{oc}", bufs=1)
                    nc.tensor.matmul(hk_ps, nh2_r, kh2_r[:, oc, :], start=True,
                                     stop=True, skip_group_check=True)
                    nc.vector.reduce_max(hkm[:, oc:oc + 1], hk_ps,
                                         axis=mybir.AxisListType.X)
                hk2_f = statp.tile([2, 1], F32)
                nc.vector.reduce_max(hk2_f, hkm, axis=mybir.AxisListType.X)
                hkt_ps = psb.tile([1, 2], F32, tag="hkt", bufs=1)
                nc.tensor.transpose(hkt_ps, hk2_f, id2_f)
                stab_sb = statp.tile([1, 1], F32)
                nc.vector.reduce_max(stab_sb, hkt_ps, axis=mybir.AxisListType.X)
                nc.sync.dma_start(out=stab_out[:, :], in_=stab_sb)

                for sc in range(2):
                    vh_ps = psb.tile([128, D_MODEL], F32, tag="vh", bufs=1)
                    for c in range(4):
                        nc.tensor.matmul(vh_ps, v_c(c)[:, ts(sc, 128)],
                                         wv_r[:, c, :], start=(c == 0),
                                         stop=(c == 3), skip_group_check=True)
                    nc.scalar.copy(vhT_sb[:, sc, :], vh_ps)
                nc.sync.dma_start(
                    out=vhT_out[:, :].rearrange("(s p) f -> p s f", p=128),
                    in_=vhT_sb)

                rsbc_ps = psb.tile([128, SLA], F32, tag="rsbc", bufs=1)
                nc.tensor.matmul(rsbc_ps, on_r, rstd_r, start=True, stop=True,
                                 skip_group_check=True)
                nc.scalar.copy(rsbc_r, rsbc_ps)

                for oc in range(4):
                    qh_ps = psb.tile([128, SLA], F32, tag="qh")
                    for c in range(4):
                        nc.tensor.matmul(qh_ps, wq_r[:, c, ts(oc, 128)],
                                         q_c(c), start=(c == 0),
                                         stop=False, skip_group_check=True)
                    nc.tensor.matmul(qh_ps, gq_r[0:1, ts(oc, 128)], mu_r,
                                     start=False, stop=True,
                                     skip_group_check=True)
                    nc.vector.tensor_mul(qh_sb[:, oc, :], qh_ps, rsbc_r)
                    nc.scalar.activation(qh_sb[:, oc, :], qh_sb[:, oc, :], IDENT,
                                         bias=cq_sb[:, oc:oc + 1], scale=1.0)
                nc.sync.dma_start(
                    out=qh_out[:, :].rearrange("(c p) f -> p c f", p=128),
                    in_=qh_sb)
    nc.compile()
    return nc


# --------------------------------------------------------------------------
# Phase B: head-sharded Performer attention + FC row-block
# --------------------------------------------------------------------------
def build_phase_b(debug=False):
    nc = bacc.Bacc("TRN2", target_bir_lowering=False, debug=False, num_devices=NC)
    khh = nc.dram_tensor("khh", [D_K, N], F32, kind="ExternalInput")
    qhh = nc.dram_tensor("qhh", [D_K, N], F32, kind="ExternalInput")
    vht = nc.dram_tensor("vht", [128, NCH * D_V], F32, kind="ExternalInput")
    rft = nc.dram_tensor("rft", [D_K, M], F32, kind="ExternalInput")
    rneg = nc.dram_tensor("rneg", [D_K, M], F32, kind="ExternalInput")
    wfc = nc.dram_tensor("wfc", [D_V, D_MODEL], F32, kind="ExternalInput")
    pairmask = nc.dram_tensor("pairmask", [C, 2 * C], F32, kind="ExternalInput")
    identm = nc.dram_tensor("identm", [128, 128], F32, kind="ExternalInput")
    onescol = nc.dram_tensor("onescol", [128, 1], F32, kind="ExternalInput")
    stabcol = nc.dram_tensor("stabcol", [128, 1], F32, kind="ExternalInput")
    c2 = nc.dram_tensor("c2", [128, 2 * NCH], F32, kind="ExternalInput")
    epsk = nc.dram_tensor("epsk", [128, NCH], F32, kind="ExternalInput")
    eps_row = nc.dram_tensor("eps_row", [1, N], F32, kind="ExternalInput")
    zeros66 = nc.dram_tensor("zeros66", [128, 3 * 66], F32, kind="ExternalInput")
    misc = nc.dram_tensor("misc", [1, 1], F32, kind="ExternalInput")  # M*eps
    out_d = nc.dram_tensor("out", [N, D_MODEL], F32, kind="ExternalOutput")

    with tile.TileContext(nc) as tc:
        with (
            tc.tile_pool(name="consts", bufs=1) as consts,
            tc.tile_pool(name="krows", bufs=1) as krows,
            tc.tile_pool(name="feat", bufs=1) as feat,
            tc.tile_pool(name="ktrp", bufs=1) as ktrp,
            tc.tile_pool(name="ktT", bufs=NCH) as ktTp,
            tc.tile_pool(name="atp", bufs=2) as atp,
            tc.tile_pool(name="ssb", bufs=NPAIR + 1) as ssbp,
            tc.tile_pool(name="post", bufs=4) as post,
            tc.tile_pool(name="outp", bufs=3) as outp,
        ):
            # ---- casting loads ----
            qhr = krows.tile([D_K, N], F32R)
            nc.gpsimd.dma_start(out=qhr, in_=qhh[:, :])
            khr = krows.tile([D_K, N], F32R)
            nc.gpsimd.dma_start(out=khr, in_=khh[:, :])
            vha = krows.tile([128, NCH, 66], F32R)
            nc.gpsimd.dma_start(
                out=vha[:, :, 0:D_V],
                in_=vht[:, :].rearrange("p (ch f) -> p ch f", ch=NCH))

            # ---- plain loads + engine conversions for small consts ----
            rft_f = consts.tile([D_K, M], F32)
            nc.sync.dma_start(out=rft_f, in_=rft[:, :])
            rft_r = consts.tile([D_K, M], F32R)
            nc.vector.tensor_copy(rft_r, rft_f)
            rneg_f = consts.tile([D_K, M], F32)
            nc.sync.dma_start(out=rneg_f, in_=rneg[:, :])
            rneg_r = consts.tile([D_K, M], F32R)
            nc.gpsimd.tensor_copy(rneg_r, rneg_f)
            wfc_f = consts.tile([D_V, D_MODEL], F32)
            nc.sync.dma_start(out=wfc_f, in_=wfc[:, :])
            wfc_r = consts.tile([D_V, D_MODEL], F32R)
            nc.vector.tensor_copy(wfc_r, wfc_f)
            id_f = consts.tile([128, 128], F32)
            nc.sync.dma_start(out=id_f, in_=identm[:, :])
            id_r = consts.tile([128, 128], F32R)
            nc.gpsimd.tensor_copy(id_r, id_f)
            onc_f = consts.tile([128, 1], F32)
            nc.sync.dma_start(out=onc_f, in_=onescol[:, :])
            onc_r = consts.tile([128, 1], F32R)
            nc.vector.tensor_copy(onc_r, onc_f)
            pmask_sb = consts.tile([C, 2 * C], F32)
            nc.sync.dma_start(out=pmask_sb, in_=pairmask[:, :])
            stab_sb = consts.tile([128, 1], F32)
            nc.sync.dma_start(out=stab_sb, in_=stabcol[:, :])
            c2_sb = consts.tile([128, 2 * NCH], F32)
            nc.sync.dma_start(out=c2_sb, in_=c2[:, :])
            epsk_sb = consts.tile([128, NCH], F32)
            nc.sync.dma_start(out=epsk_sb, in_=epsk[:, :])
            z66_sb = consts.tile([128, 3, 66], F32)
            nc.sync.dma_start(
                out=z66_sb, in_=zeros66[:, :].rearrange("p (a b) -> p a b", a=3))
            misc_sb = consts.tile([1, 1], F32)
            nc.sync.dma_start(out=misc_sb, in_=misc[:, :])

            # vha constant columns: 64 -> 1.0, 65 -> 0.0
            nc.gpsimd.tensor_copy(
                vha[:, :, D_V:D_V + 2],
                c2_sb[:, :].rearrange("p (ch f) -> p ch f", ch=NCH))

            # kh^2 rows (squares of the casting-DMA'd kh)
            kh2r = krows.tile([D_K, N], F32R)
            for s in range(NSL):
                nc.vector.tensor_mul(kh2r[:, ts(s, SL)], khr[:, ts(s, SL)],
                                     khr[:, ts(s, SL)])

            # ---- q~ features [m-major: 128 x 3 x N] ----
            qt_feat = feat.tile([128, 3, N], F32R)
            nc.gpsimd.dma_start(out=qt_feat[11:12, 2, :], in_=eps_row[:, :])
            with (
                tc.tile_pool(name="psqp", bufs=2, space="PSUM") as psqp,
                tc.tile_pool(name="pssp", bufs=2, space="PSUM") as pssp,
                tc.tile_pool(name="qtmp2", bufs=2) as qtmp2,
            ):
                for half in range(2):       # two 1024-wide exp batches per mc
                    for mc in range(3):
                        mrows = 128 if mc < 2 else 10
                        qp_ps = psqp.tile([128, 2 * SL], F32, tag="qp")
                        for sub in range(2):
                            s = 2 * half + sub
                            nc.tensor.matmul(
                                qp_ps[0:mrows, ts(sub, SL)],
                                rft_r[:, ds(mc * 128, mrows)],
                                qhr[:, ts(s, SL)], start=True, stop=True,
                                skip_group_check=True)
                        nc.scalar.activation(
                            qt_feat[0:mrows, mc, ts(half, 2 * SL)],
                            qp_ps[0:mrows, :], EXP, bias=0.0, scale=1.0)
                for s in range(NSL):
                    sp_ps = pssp.tile([1, SL], F32, tag="sp")
                    nc.tensor.matmul(sp_ps, onc_r, qt_feat[:, 0, ts(s, SL)],
                                     start=True, stop=False, skip_group_check=True)
                    nc.tensor.matmul(sp_ps, onc_r, qt_feat[:, 1, ts(s, SL)],
                                     start=False, stop=False, skip_group_check=True)
                    nc.tensor.matmul(sp_ps, onc_r[0:10, :],
                                     qt_feat[0:10, 2, ts(s, SL)],
                                     start=False, stop=True, skip_group_check=True)
                    sp_sb = qtmp2.tile([1, SL], F32, tag="sp_sb")
                    nc.scalar.activation(sp_sb, sp_ps, IDENT,
                                         bias=misc_sb[0:1, 0:1], scale=1.0)
                    nc.gpsimd.dma_start(out=qt_feat[10:11, 2, ts(s, SL)],
                                        in_=sp_sb)

            # ---- stage 1: k~ features, z accumulation, transposes,
            #      then (per pair) attention blocks + state scan ----
            ktr = ktrp.tile([128, NCH, 268], F32R)
            nc.vector.tensor_copy(
                ktr[:, :, 266:267],
                epsk_sb[:, :].rearrange("p (ch f) -> p ch f", ch=NCH))

            s_tiles = [ssbp.tile([128, 3, 66], F32R, tag="ssb", name=f"ssb{i}")
                       for i in range(NPAIR + 1)]
            nc.vector.tensor_copy(s_tiles[0][:, 0:2, :], z66_sb[:, 0:2, :])
            nc.scalar.copy(s_tiles[0][0:12, 2:3, :], z66_sb[0:12, 2:3, :])

            ktT_tiles = {}
            at1_list = []
            at2_list = []
            zcol2 = feat.tile([128, 3, 2], F32R)
            nc.vector.tensor_copy(zcol2[:, :, 1:2], z66_sb[:, :, 0:1])
            with (
                tc.tile_pool(name="pskp", bufs=2, space="PSUM") as pskp,
                tc.tile_pool(name="pstr", bufs=1, space="PSUM") as pstr,
                tc.tile_pool(name="psat", bufs=2, space="PSUM") as psat,
                tc.tile_pool(name="pssd", bufs=2, space="PSUM") as pssd,
            ):
                for ch in range(NCH):
                    # k features for chunk ch
                    kp_ps = pskp.tile([C, M], F32, tag="kp")
                    nc.tensor.matmul(kp_ps, khr[:, ts(ch, C)], rft_r,
                                     start=True, stop=False, skip_group_check=True)
                    nc.tensor.matmul(kp_ps, kh2r[:, ts(ch, C)], rneg_r,
                                     start=False, stop=True, skip_group_check=True)
                    with nc.allow_low_precision(reason="fp32r accum ~ fp32"):
                        nc.scalar.activation(
                            ktr[:, ch, 0:M], kp_ps, EXP,
                            bias=stab_sb[:, 0:1], scale=1.0,
                            accum_out=ktr[:, ch, 267:268])
                    # transpose the chunk's features to m-major
                    tp_ps = pstr.tile([128, 3, 128], F32R, tag="tp")
                    nc.tensor.transpose(tp_ps[:, 0, :], ktr[:, ch, 0:128], id_r)
                    nc.tensor.transpose(tp_ps[:, 1, :], ktr[:, ch, 128:256], id_r)
                    nc.tensor.transpose(tp_ps[0:12, 2, :], ktr[:, ch, 256:268], id_r)
                    ktT = ktTp.tile([128, 3, C], F32R, tag="ktT", name=f"ktT{ch}")
                    if ch % 2 == 0:
                        nc.vector.tensor_copy(ktT[:, 0:2, :], tp_ps[:, 0:2, :])
                        nc.scalar.copy(ktT[0:12, 2:3, :], tp_ps[0:12, 2:3, :])
                    else:
                        nc.scalar.copy(ktT[:, 0:2, :], tp_ps[:, 0:2, :])
                        nc.vector.tensor_copy(ktT[0:12, 2:3, :], tp_ps[0:12, 2:3, :])
                    ktT_tiles[ch] = ktT

                    if ch % 2 == 1:
                        i = ch // 2
                        a, b = 2 * i, 2 * i + 1
                        # pair attention block: keys a x queries {a, b}
                        at1_ps = psat.tile([C, 2 * C], F32, tag="at1")
                        ka = ktT_tiles[a]
                        for mc in range(2):
                            nc.tensor.matmul(at1_ps, ka[:, mc, :],
                                             qt_feat[:, mc, ts(i, 2 * C)],
                                             start=(mc == 0), stop=False,
                                             skip_group_check=True)
                        nc.tensor.matmul(at1_ps, ka[0:12, 2, :],
                                         qt_feat[0:12, 2, ts(i, 2 * C)],
                                         start=False, stop=True,
                                         skip_group_check=True)
                        at1_r = atp.tile([C, 2 * C], F32R, tag="at1r",
                                         name=f"at1r{i}", bufs=NPAIR)
                        nc.vector.tensor_mul(at1_r, at1_ps, pmask_sb)
                        at1_list.append(at1_r)
                        # odd diag block: keys b x queries b
                        at2_ps = psat.tile([C, C], F32, tag="at2", bufs=1)
                        kb = ktT_tiles[b]
                        for mc in range(2):
                            nc.tensor.matmul(at2_ps, kb[:, mc, :],
                                             qt_feat[:, mc, ts(b, C)],
                                             start=(mc == 0), stop=False,
                                             skip_group_check=True)
                        nc.tensor.matmul(at2_ps, kb[0:12, 2, :],
                                         qt_feat[0:12, 2, ts(b, C)],
                                         start=False, stop=True,
                                         skip_group_check=True)
                        at2_r = atp.tile([C, C], F32R, tag="at2r",
                                         name=f"at2r{i}", bufs=NPAIR)
                        nc.vector.tensor_mul(at2_r, at2_ps, pmask_sb[:, 0:C])
                        at2_list.append(at2_r)

                        # state update for the pair
                        sd_ps = pssd.tile([128, 3, 66], F32, tag="sd")
                        for mc, cols in ((0, (0, 128)), (1, (128, 256)),
                                         (2, (256, 268))):
                            dst = sd_ps[:, mc, :] if mc < 2 else sd_ps[0:12, 2, :]
                            for j, ch2 in enumerate((a, b)):
                                nc.tensor.matmul(dst, ktr[:, ch2, cols[0]:cols[1]],
                                                 vha[:, ch2, 0:66], start=(j == 0),
                                                 stop=(j == 1),
                                                 skip_group_check=True)
                        nc.vector.tensor_add(s_tiles[i + 1][:, 0:2, 0:66],
                                             s_tiles[i][:, 0:2, 0:66],
                                             sd_ps[:, 0:2, :])
                        nc.vector.tensor_add(s_tiles[i + 1][0:12, 2:3, 0:66],
                                             s_tiles[i][0:12, 2:3, 0:66],
                                             sd_ps[0:12, 2:3, :])

            # z column = final state's column 64 (already column-major)
            zfin = s_tiles[NPAIR]
            nc.vector.tensor_copy(zcol2[:, 0:2, 0:1], zfin[:, 0:2, 64:65])
            nc.scalar.copy(zcol2[0:12, 2:3, 0:1], zfin[0:12, 2:3, 64:65])

            # ---- per-chunk output, normalize, FC ----
            with (
                tc.tile_pool(name="pso", bufs=3, space="PSUM") as pso,
                tc.tile_pool(name="psfc", bufs=2, space="PSUM") as psfc,
                tc.tile_pool(name="pstr2", bufs=2, space="PSUM") as pstr2,
            ):
                for ch in range(NCH):
                    i = ch // 2
                    o_ps = pso.tile([C, 68], F32, tag="o")
                    for mc in range(2):
                        nc.tensor.matmul(o_ps[:, 0:66], qt_feat[:, mc, ts(ch, C)],
                                         s_tiles[i][:, mc, 0:66],
                                         start=(mc == 0), stop=False,
                                         skip_group_check=True)
                    nc.tensor.matmul(o_ps[:, 0:66], qt_feat[0:12, 2, ts(ch, C)],
                                     s_tiles[i][0:12, 2, 0:66],
                                     start=False, stop=False, skip_group_check=True)
                    if ch % 2 == 0:
                        nc.tensor.matmul(
                            o_ps[:, 0:66], at1_list[i][:, 0:C],
                            vha[:, ch, :], start=False, stop=True,
                            skip_group_check=True)
                    else:
                        nc.tensor.matmul(
                            o_ps[:, 0:66], at1_list[i][:, C:2 * C],
                            vha[:, ch - 1, :], start=False, stop=False,
                            skip_group_check=True)
                        nc.tensor.matmul(
                            o_ps[:, 0:66], at2_list[i],
                            vha[:, ch, :], start=False, stop=True,
                            skip_group_check=True)
                    # d = q~ . z in columns 66:68 (col 67 is a zero pad)
                    for mc in range(2):
                        nc.tensor.matmul(o_ps[:, 66:68], qt_feat[:, mc, ts(ch, C)],
                                         zcol2[:, mc, :], start=(mc == 0),
                                         stop=False, skip_group_check=True)
                    nc.tensor.matmul(o_ps[:, 66:68], qt_feat[0:12, 2, ts(ch, C)],
                                     zcol2[0:12, 2, :], start=False, stop=True,
                                     skip_group_check=True)
                    dcols = post.tile([C, 2, 2], F32, tag="dcols")
                    nc.vector.tensor_copy(
                        dcols, o_ps[:, 64:68].rearrange("p (a b) -> p a b", a=2))
                    dd = post.tile([C, 1], F32, tag="dd")
                    nc.vector.tensor_mul(dd, dcols[:, 0, 0:1], dcols[:, 1, 0:1])
                    rec = post.tile([C, 1], F32, tag="rec")
                    nc.vector.reciprocal(rec, dd)
                    attn_r = post.tile([C, D_V], F32R, tag="attn")
                    nc.vector.tensor_scalar_mul(attn_r, o_ps[:, 0:D_V], rec)
                    tr_ps = pstr2.tile([D_V, C], F32R, tag="tr")
                    nc.tensor.transpose(tr_ps, attn_r, id_r)
                    attnT_r = post.tile([D_V, C], F32R, tag="attnT")
                    if ch % 2 == 0:
                        nc.scalar.copy(attnT_r, tr_ps)
                    else:
                        nc.vector.tensor_copy(attnT_r, tr_ps)
                    fc_ps = psfc.tile([C, D_MODEL], F32, tag="fc")
                    nc.tensor.matmul(fc_ps, attnT_r, wfc_r, start=True, stop=True,
                                     skip_group_check=True)
                    o_sb = outp.tile([C, D_MODEL], F32, tag="osb")
                    if ch % 2 == 0:
                        nc.vector.tensor_copy(o_sb, fc_ps)
                    else:
                        nc.scalar.copy(o_sb, fc_ps)
                    nc.sync.dma_start(out=out_d[ts(ch, C), :], in_=o_sb)
    nc.compile()
    return nc


# --------------------------------------------------------------------------
# Host orchestration
# --------------------------------------------------------------------------
_CACHE = {}


def _get_programs():
    if "a" not in _CACHE:
        _CACHE["a"] = build_phase_a()
        _CACHE["b"] = build_phase_b()
    return _CACHE["a"], _CACHE["b"]


def _prep_a_maps(q, k, v, Wq, Wk, Wv, gamma, beta):
    qT = np.ascontiguousarray(q[0].T)
    kT = np.ascontiguousarray(k[0].T)
    vT = np.ascontiguousarray(v[0].T)
    Wqe = np.ascontiguousarray((gamma[:, None] * Wq) * SCALE)
    Wke = np.ascontiguousarray(Wk * SCALE)
    cq_all = (beta @ Wq) * SCALE                       # [512]
    cq = np.ascontiguousarray(cq_all.reshape(4, 128).T)
    wmean = np.full((128, 1), 1.0 / D_MODEL, np.float32)
    nh2 = np.zeros((128, 2), np.float32)
    nh2[0:64, 0] = -0.5
    nh2[64:128, 1] = -0.5
    ones_r = np.ones((1, 128), np.float32)
    gqneg = np.ascontiguousarray(-Wqe.sum(axis=0)[None, :])
    ident2 = np.eye(2, dtype=np.float32)
    misc = np.full((1, 1), LN_EPS, np.float32)
    in_a = []
    for j in range(NC):
        sl = slice(j * SLA, (j + 1) * SLA)
        xs = np.concatenate([qT[:, sl], kT[:, sl], vT[:, sl]], axis=1)
        in_a.append({
            "xs": np.ascontiguousarray(xs),
            "Wqe": Wqe, "Wke": Wke, "Wv": np.ascontiguousarray(Wv),
            "cq": cq, "wmean": wmean, "nh2": nh2, "ones_r": ones_r,
            "gqneg": gqneg, "ident2": ident2, "misc": misc,
        })
    return in_a


def _prep_b_maps(W_fc, rf, res_a):
    kh_full = np.concatenate([r["kh"] for r in res_a], axis=1)   # [512, N]
    qh_full = np.concatenate([r["qh"] for r in res_a], axis=1)   # [512, N]
    vh_full = np.concatenate([r["vhT"] for r in res_a], axis=0)  # [N, 512]
    k_stab = np.float32(max(r["stab"][0, 0] for r in res_a))

    rftT = np.ascontiguousarray(rf.T)                  # [64, 266]
    rneg = np.full((D_K, M), -0.5, np.float32)
    tri = np.triu(np.ones((C, C), np.float32))
    pairmask = np.ascontiguousarray(
        np.concatenate([tri, np.ones((C, C), np.float32)], axis=1))
    identm = np.eye(128, dtype=np.float32)
    onescol = np.ones((128, 1), np.float32)
    stabcol = np.full((128, 1), -k_stab, np.float32)
    c2 = np.zeros((128, 2 * NCH), np.float32)
    c2[:, 0::2] = 1.0                                  # vha col 64 = 1, col 65 = 0
    epsk = np.full((128, NCH), KERNEL_EPS, np.float32)
    eps_row = np.full((1, N), KERNEL_EPS, np.float32)
    zeros66 = np.zeros((128, 3 * 66), np.float32)
    misc = np.full((1, 1), M * KERNEL_EPS, np.float32)

    in_b = []
    for h in range(NC):
        rows = slice(h * D_K, (h + 1) * D_K)
        vh_h = vh_full[:, h * D_V:(h + 1) * D_V]       # [N, 64]
        vht = np.ascontiguousarray(
            vh_h.reshape(NCH, 128, D_V).transpose(1, 0, 2).reshape(128, NCH * D_V))
        in_b.append({
            "khh": np.ascontiguousarray(kh_full[rows]),
            "qhh": np.ascontiguousarray(qh_full[rows]),
            "vht": vht,
            "rft": rftT, "rneg": rneg,
            "wfc": np.ascontiguousarray(W_fc[rows, :] * float(M)),
            "pairmask": pairmask, "identm": identm, "onescol": onescol,
            "stabcol": stabcol, "c2": c2, "epsk": epsk, "eps_row": eps_row,
            "zeros66": zeros66, "misc": misc,
        })
    return in_b


def _cast_all(*arrs):
    return [np.asarray(a, np.float32) for a in arrs]


def kernel(q, k, v, Wq, Wk, Wv, W_fc, b_fc, gamma, beta, rf):
    q, k, v, Wq, Wk, Wv, W_fc, b_fc, gamma, beta, rf = _cast_all(
        q, k, v, Wq, Wk, Wv, W_fc, b_fc, gamma, beta, rf)

    nc_a, nc_b = _get_programs()
    cores = list(range(NC))

    in_a = _prep_a_maps(q, k, v, Wq, Wk, Wv, gamma, beta)
    res_a = run_bass_kernel_spmd(nc_a, in_a, core_ids=cores)

    in_b = _prep_b_maps(W_fc, rf, res_a.results)
    res_b = run_bass_kernel_spmd(nc_b, in_b, core_ids=cores)

    out = np.zeros((N, D_MODEL), np.float32)
    for r in res_b.results:
        out += r["out"]
    out += b_fc[None, :]
    out += q[0]
    return out[None].astype(np.float32)


def trace_args(inputs):
    """For test.py: returns [(phase, nc, in_maps), ...] re-runnable with trace."""
    q, k, v, Wq, Wk, Wv, W_fc, b_fc, gamma, beta, rf = _cast_all(
        inputs["q"], inputs["k"], inputs["v"], inputs["Wq"], inputs["Wk"],
        inputs["Wv"], inputs["W_fc"], inputs["b_fc"], inputs["gamma"],
        inputs["beta"], inputs["rf"])
    nc_a, nc_b = _get_programs()
    in_a = _prep_a_maps(q, k, v, Wq, Wk, Wv, gamma, beta)
    res_a = run_bass_kernel_spmd(nc_a, in_a, core_ids=list(range(NC)))
    in_b = _prep_b_maps(W_fc, rf, res_a.results)
    return [("a", nc_a, in_a), ("b", nc_b, in_b)]


if __name__ == "__main__":
    rng = np.random.default_rng(0)
    inputs = {
        "q": rng.standard_normal((1, N, D_MODEL)).astype(np.float32),
        "k": rng.standard_normal((1, N, D_MODEL)).astype(np.float32),
        "v": rng.standard_normal((1, N, D_MODEL)).astype(np.float32),
        "Wq": (rng.standard_normal((D_MODEL, 512)) * 0.04).astype(np.float32),
        "Wk": (rng.standard_normal((D_MODEL, 512)) * 0.04).astype(np.float32),
        "Wv": (rng.standard_normal((D_MODEL, 512)) * 0.04).astype(np.float32),
        "W_fc": (rng.standard_normal((512, D_MODEL)) * 0.04).astype(np.float32),
        "b_fc": np.zeros(D_MODEL, np.float32),
        "gamma": np.ones(D_MODEL, np.float32),
        "beta": np.zeros(D_MODEL, np.float32),
        "rf": rng.standard_normal((M, D_K)).astype(np.float32),
    }
    out = kernel(**inputs)
    print("kernel output", out.shape, out.dtype)
